# revision 1
# baseline (speedup 1.0000x reference)
import math
import functools

import jax
import jax.numpy as jnp
import numpy as np

# nn_CAM co-attention model, hardcoded shapes.
B, T, D_IN, D_ENC = 4096, 8, 512, 128
N_CORES = 8
B_SHARD = B // N_CORES  # 512 samples per core

_SCALE = 1.0 / math.sqrt(2 * D_ENC)


def _forward(f1, f2, W_e1, b_e1, W_e2, b_e2, Wa_aff, Wv_aff,
             W_a, W_v, W_ca, W_cv, W_ha, W_hv, W_r1, b_r1, W_r2, b_r2):
    # f1/f2: [Bs, T, D_IN] on one core
    aud = f1 @ W_e1.T + b_e1            # [Bs,T,128]
    vis = f2 @ W_e2.T + b_e2
    av = jnp.concatenate([aud, vis], axis=-1)   # [Bs,T,256]
    avT = jnp.swapaxes(av, 1, 2)                # [Bs,256,T]
    audT = jnp.swapaxes(aud, 1, 2)              # [Bs,128,T]
    visT = jnp.swapaxes(vis, 1, 2)
    a_t = avT @ Wa_aff.T                        # [Bs,256,T]
    att_a = jnp.tanh((audT @ jnp.swapaxes(a_t, 1, 2)) * _SCALE)
    v_t = avT @ Wv_aff.T
    att_v = jnp.tanh((visT @ jnp.swapaxes(v_t, 1, 2)) * _SCALE)
    H_a = jax.nn.relu(att_a @ W_ca.T + audT @ W_a.T)   # [Bs,128,32]
    H_v = jax.nn.relu(att_v @ W_cv.T + visT @ W_v.T)
    att_aud_f = jnp.swapaxes(H_a @ W_ha.T, 1, 2) + aud  # [Bs,T,128]
    att_vis_f = jnp.swapaxes(H_v @ W_hv.T, 1, 2) + vis
    avf = jnp.concatenate([att_aud_f, att_vis_f], axis=-1)  # [Bs,T,256]
    # r1/r2 are linear back-to-back (dropout is identity in eval):
    # collapse into a single [256,1] projection to cut two matmuls.
    w = W_r1.T @ W_r2.T                 # [256,1]
    c0 = b_r1 @ W_r2.T + b_r2           # [1]
    out = avf @ w + c0                  # [Bs,T,1]
    return out


_pmapped = jax.pmap(
    _forward,
    axis_name="x",
    in_axes=(0, 0) + (None,) * 16,
    devices=jax.devices()[:N_CORES],
)


def kernel(f1_norm, f2_norm, W_e1, b_e1, W_e2, b_e2, Wa_aff, Wv_aff,
           W_a, W_v, W_ca, W_cv, W_ha, W_hv, W_r1, b_r1, W_r2, b_r2):
    f1 = np.asarray(f1_norm, dtype=np.float32).reshape(N_CORES, B_SHARD, T, D_IN)
    f2 = np.asarray(f2_norm, dtype=np.float32).reshape(N_CORES, B_SHARD, T, D_IN)
    out = _pmapped(
        f1, f2,
        jnp.asarray(W_e1), jnp.asarray(b_e1),
        jnp.asarray(W_e2), jnp.asarray(b_e2),
        jnp.asarray(Wa_aff), jnp.asarray(Wv_aff),
        jnp.asarray(W_a), jnp.asarray(W_v),
        jnp.asarray(W_ca), jnp.asarray(W_cv),
        jnp.asarray(W_ha), jnp.asarray(W_hv),
        jnp.asarray(W_r1), jnp.asarray(b_r1),
        jnp.asarray(W_r2), jnp.asarray(b_r2),
    )
    return np.asarray(out).reshape(B, T, 1).astype(np.float32)

